# revision 10
# baseline (speedup 1.0000x reference)
"""Segment-max aggregation of gathered embedding rows (NodeMaxAggregator).

out[n, :] = max_{e : segment_ids[e]==n} table[hyperedge_ids[e], :]

Strategy (8 NeuronCores, data-parallel over node segments):
  - Host: pad each node's entry list to K columns (pad index -> an extra
    -inf row appended to the table), split nodes contiguously across the
    8 cores, and lay the indices out per [partition, tile, k].
  - Device (per core): for each tile of 128 nodes, one indirect DMA
    gathers 128*K rows (512B each) from the table in HBM into SBUF
    (partition p holds node p's K rows back to back), then a DVE
    tensor_max tree reduces K -> 1, and the [128, D] result is DMA'd to
    the output.
"""

import math
from contextlib import ExitStack

import numpy as np

P = 128  # SBUF partitions
N_CORES = 8

_PROGRAM_CACHE: dict = {}
LAST_RESULT = None  # BassKernelResults of the most recent kernel() call


def _build_program(n_rows_aug: int, d: int, k: int, n_tiles: int):
    """One SPMD Bass program; per-core data differences live in the inputs."""
    import concourse.bass as bass
    import concourse.mybir as mybir
    import concourse.tile as tile
    from concourse import bacc

    f32 = mybir.dt.float32
    i32 = mybir.dt.int32

    idx_cols = n_tiles * k

    nc = bacc.Bacc(None, target_bir_lowering=False)
    table = nc.dram_tensor("table", [n_rows_aug, d], f32, kind="ExternalInput")
    idx = nc.dram_tensor("idx", [P, idx_cols], i32, kind="ExternalInput")
    out = nc.dram_tensor("out", [n_tiles * P, d], f32, kind="ExternalOutput")

    with ExitStack() as ctx:
        tc = ctx.enter_context(tile.TileContext(nc))
        idx_pool = ctx.enter_context(tc.tile_pool(name="idxp", bufs=1))
        g_pool = ctx.enter_context(tc.tile_pool(name="g", bufs=3))
        r_pool = ctx.enter_context(tc.tile_pool(name="r", bufs=2))

        idx_sb = idx_pool.tile([P, idx_cols], i32)
        nc.sync.dma_start(idx_sb[:], idx[:])

        for t in range(n_tiles):
            g = g_pool.tile([P, k * d], f32)
            # HW SWDGE reads ONE index per partition per indirect DMA: each
            # descriptor copies the dest's free extent contiguously from
            # table[idx[p]]. So issue k gathers of [P, d] slices.
            for j in range(k):
                nc.gpsimd.indirect_dma_start(
                    out=g[:, j * d : (j + 1) * d],
                    out_offset=None,
                    in_=table[:],
                    in_offset=bass.IndirectOffsetOnAxis(
                        ap=idx_sb[:, t * k + j : t * k + j + 1],
                        axis=0,
                    ),
                )
            # tree max over the k gathered rows per partition
            cur = g
            width = k
            while width > 1:
                lo = width // 2
                hi = width - lo  # if odd, max(first hi, last hi) overlaps mid
                nxt = r_pool.tile([P, hi * d], f32, tag=f"r{hi}")
                nc.vector.tensor_max(
                    nxt[:], cur[:, : hi * d], cur[:, (width - hi) * d :]
                )
                cur = nxt
                width = hi
            nc.sync.dma_start(out[t * P : (t + 1) * P, :], cur[:])

    nc.finalize()
    return nc


def _get_program(key):
    if key not in _PROGRAM_CACHE:
        _PROGRAM_CACHE[key] = _build_program(*key)
    return _PROGRAM_CACHE[key]


def _pad_entries(hyperedge_ids: np.ndarray, segment_ids: np.ndarray, n_nodes: int):
    """[n_nodes, K] int32 index matrix, padded with the -inf row index."""
    e_total = hyperedge_ids.shape[0]
    counts = np.bincount(segment_ids, minlength=n_nodes).astype(np.int64)
    k = int(counts.max()) if e_total else 1
    pad_row = -1  # filled in by caller (row index of the -inf row)
    starts = np.zeros(n_nodes, dtype=np.int64)
    np.cumsum(counts[:-1], out=starts[1:])
    rank = np.arange(e_total, dtype=np.int64) - starts[segment_ids]
    idx = np.full((n_nodes, k), pad_row, dtype=np.int64)
    idx[segment_ids, rank] = hyperedge_ids
    return idx, k


def kernel(table, hyperedge_ids, segment_ids, n_nodes):
    from concourse.bass_utils import run_bass_kernel_spmd

    n_nodes = int(n_nodes)
    table = np.ascontiguousarray(np.asarray(table, dtype=np.float32))
    hyperedge_ids = np.asarray(hyperedge_ids)
    segment_ids = np.asarray(segment_ids)
    n_rows, d = table.shape

    # augmented table: one extra very-negative row as the padding target
    # (finite so simulators' nonfinite checks stay quiet; any real entry wins)
    table_aug = np.vstack([table, np.full((1, d), -3.0e38, dtype=np.float32)])
    pad_row = n_rows

    idx_padded, k = _pad_entries(hyperedge_ids, segment_ids, n_nodes)
    idx_padded[idx_padded < 0] = pad_row

    # split nodes across cores; pad each core's share to a multiple of P
    n_per_core = math.ceil(n_nodes / N_CORES)
    n_tiles = math.ceil(n_per_core / P)
    n_pc_pad = n_tiles * P

    nc = _get_program((n_rows + 1, d, k, n_tiles))

    in_maps = []
    for c in range(N_CORES):
        lo = min(c * n_per_core, n_nodes)
        hi = min(lo + n_per_core, n_nodes)
        block = np.full((n_pc_pad, k), pad_row, dtype=np.int64)
        block[: hi - lo] = idx_padded[lo:hi]
        # [node, k] -> [partition, tile, k]: node = t*P + p
        arranged = (
            block.reshape(n_tiles, P, k).transpose(1, 0, 2).reshape(P, n_tiles * k)
        )
        in_maps.append(
            {
                "table": table_aug,
                "idx": np.ascontiguousarray(arranged.astype(np.int32)),
            }
        )

    res = run_bass_kernel_spmd(nc, in_maps, core_ids=list(range(N_CORES)))
    global LAST_RESULT
    LAST_RESULT = res

    parts = []
    for c in range(N_CORES):
        lo = min(c * n_per_core, n_nodes)
        hi = min(lo + n_per_core, n_nodes)
        parts.append(res.results[c]["out"][: hi - lo])
    return np.concatenate(parts, axis=0)
